# revision 1
# baseline (speedup 1.0000x reference)
"""Segment mean-pool (global_mean_pool) kernel for Trainium2, 8 NeuronCores.

Problem: x [1_000_000, 256] f32, batch [1_000_000] sorted int in [0, 1024).
Output [1024, 256]: per-segment mean of rows of x.

Strategy
--------
batch is sorted, so each segment is a contiguous row range. We shard by
*segment boundaries*: core k owns the 128 segments [128k, 128k+128) and the
contiguous rows belonging to them. Each core computes its 128 output rows
fully on-device, so no collective is needed; the host just concatenates the
eight [128, 256] results.

Per core, rows are streamed in 128-row chunks. For each chunk the device
builds a one-hot routing matrix hot[i, s] = (batch_local[row i] == s) with a
single VectorE tensor_scalar(is_equal) against a resident iota constant, and
TensorE accumulates hot.T @ x_chunk into a PSUM tile [128 segs, 257]. Column
256 of x is a host-appended ones column, so segment counts fall out of the
same matmul. The mean is sums * reciprocal(max(count, 1)) on VectorE.

Pad rows (to make every core the same fixed number of chunks) carry
batch_local = -1, which matches no one-hot column and contributes nothing.

All heavy data movement (the ~1 GB of x) and all row arithmetic happen
on-device; the host only does index bookkeeping (searchsorted), padding, and
layout.
"""

import math

import numpy as np

P = 128           # SBUF partitions / rows per chunk
F = 256           # feature dim
FC = F + 1        # features + ones column (for counts)
G = 1024          # total segments
NCORES = 8
SEG_PER_CORE = G // NCORES   # 128 segments owned by each core
CPT = 8           # chunks coalesced per DMA tile (~1.03 MB per DMA)

_cache: dict[int, object] = {}


def _build(nchunk: int):
    """Build + compile the single-core Bass program (same on all 8 cores)."""
    import concourse.mybir as mybir
    import concourse.tile as tile
    from concourse import bacc

    ntile = nchunk // CPT
    nc = bacc.Bacc("TRN2", target_bir_lowering=False, debug=False)

    x = nc.dram_tensor("x", [nchunk * P, FC], mybir.dt.float32, kind="ExternalInput").ap()
    b_t = nc.dram_tensor("b_t", [P, nchunk], mybir.dt.float32, kind="ExternalInput").ap()
    iota_c = nc.dram_tensor("iota_c", [P, SEG_PER_CORE], mybir.dt.float32, kind="ExternalInput").ap()
    out = nc.dram_tensor("out", [SEG_PER_CORE, F], mybir.dt.float32, kind="ExternalOutput").ap()

    x_r = x.rearrange("(n p) f -> p n f", p=P)   # [P, nchunk, FC]

    with tile.TileContext(nc) as tc:
        with (
            tc.tile_pool(name="xpool", bufs=4) as xpool,
            tc.tile_pool(name="hotpool", bufs=8) as hotpool,
            tc.tile_pool(name="cpool", bufs=1) as cpool,
            tc.tile_pool(name="opool", bufs=1) as opool,
            tc.tile_pool(name="psum", bufs=1, space="PSUM") as psum_pool,
        ):
            bt_sb = cpool.tile([P, nchunk], mybir.dt.float32)
            nc.sync.dma_start(bt_sb[:], b_t[:])
            iota_sb = cpool.tile([P, SEG_PER_CORE], mybir.dt.float32)
            nc.sync.dma_start(iota_sb[:], iota_c[:])

            acc = psum_pool.tile([SEG_PER_CORE, FC], mybir.dt.float32, space="PSUM")

            for t in range(ntile):
                xt = xpool.tile([P, CPT, FC], mybir.dt.float32)
                nc.sync.dma_start(xt[:], x_r[:, t * CPT : (t + 1) * CPT, :])
                for j in range(CPT):
                    c = t * CPT + j
                    hot = hotpool.tile([P, SEG_PER_CORE], mybir.dt.float32)
                    nc.vector.tensor_scalar(
                        out=hot[:],
                        in0=iota_sb[:],
                        scalar1=bt_sb[:, c : c + 1],
                        scalar2=None,
                        op0=mybir.AluOpType.is_equal,
                    )
                    nc.tensor.matmul(
                        out=acc[:],
                        lhsT=hot[:],
                        rhs=xt[:, j, :],
                        start=(c == 0),
                        stop=(c == nchunk - 1),
                    )

            cnt = opool.tile([SEG_PER_CORE, 1], mybir.dt.float32)
            nc.vector.tensor_scalar_max(cnt[:], acc[:, F : F + 1], 1.0)
            recip = opool.tile([SEG_PER_CORE, 1], mybir.dt.float32)
            nc.vector.reciprocal(recip[:], cnt[:])
            res = opool.tile([SEG_PER_CORE, F], mybir.dt.float32)
            nc.vector.tensor_scalar_mul(res[:], acc[:, :F], recip[:])
            nc.sync.dma_start(out[:], res[:])

    nc.compile()
    return nc


def _compiled(nchunk: int):
    if nchunk not in _cache:
        _cache[nchunk] = _build(nchunk)
    return _cache[nchunk]


def make_in_maps(x: np.ndarray, batch: np.ndarray):
    """Host-side shard/pad/layout. Returns (in_maps, nchunk)."""
    x = np.asarray(x, dtype=np.float32)
    batch_i = np.asarray(batch).astype(np.int64, copy=False)
    n = x.shape[0]
    assert x.shape == (n, F) and batch_i.shape == (n,)

    off = np.searchsorted(batch_i, np.arange(G + 1), side="left")
    core_off = off[:: SEG_PER_CORE]            # [NCORES + 1] row boundaries
    rows = np.diff(core_off)
    nchunk = math.ceil(rows.max() / P)
    nchunk = ((nchunk + CPT - 1) // CPT) * CPT

    iota_np = np.tile(np.arange(SEG_PER_CORE, dtype=np.float32), (P, 1))

    in_maps = []
    for k in range(NCORES):
        s, e = int(core_off[k]), int(core_off[k + 1])
        nreal = e - s
        xpad = np.zeros((nchunk * P, FC), np.float32)
        xpad[:nreal, :F] = x[s:e]
        xpad[:nreal, F] = 1.0
        b = np.full((nchunk * P,), -1.0, np.float32)
        b[:nreal] = (batch_i[s:e] - k * SEG_PER_CORE).astype(np.float32)
        in_maps.append(
            {"x": xpad, "b_t": np.ascontiguousarray(b.reshape(nchunk, P).T), "iota_c": iota_np}
        )
    return in_maps, nchunk


def run_spmd(in_maps, nchunk, **kwargs):
    from concourse.bass_utils import run_bass_kernel_spmd

    nc = _compiled(nchunk)
    return run_bass_kernel_spmd(nc, in_maps, core_ids=list(range(NCORES)), **kwargs)


def kernel(x: np.ndarray, batch: np.ndarray) -> np.ndarray:
    in_maps, nchunk = make_in_maps(x, batch)
    res = run_spmd(in_maps, nchunk)
    return np.concatenate([res.results[k]["out"] for k in range(NCORES)], axis=0)


# revision 7
# speedup vs baseline: 1.1468x; 1.1468x over previous
"""Segment mean-pool (global_mean_pool) kernel for Trainium2, 8 NeuronCores.

Problem: x [1_000_000, 256] f32, batch [1_000_000] sorted int in [0, 1024).
Output [1024, 256]: per-segment mean of rows of x.

Strategy
--------
batch is sorted, so each segment is a contiguous row range. We shard by
*segment boundaries*: core k owns the 128 segments [128k, 128k+128) and the
contiguous rows belonging to them. Each core computes its 128 output rows
fully on-device, so no collective is needed; the host just concatenates the
eight [128, 256] results.

Per core, rows are streamed in 128-row chunks. For each chunk the device
builds a one-hot routing matrix hot[i, s] = (batch_local[row i] == s) with a
single VectorE tensor_scalar(is_equal) against a resident iota constant, and
TensorE accumulates hot.T @ x_chunk into a PSUM tile [128 segs, 256].

Precision/throughput trick: fp32 matmuls cost 4 PE cycles per column. So the
host splits x into a bf16 hi/lo pair (hi = bf16(x), lo = bf16(x - hi); same
4 bytes/element as fp32, ~17 effective mantissa bits), and each chunk does
two bf16 matmuls (1 cycle/column) with the same exact 0/1 one-hot,
accumulating both into the same fp32 PSUM tile. Result matches the fp32
reference to ~1e-6 relative error at 4x the PE throughput of fp32.

The hi/lo pair is interleaved per chunk in a [ntile, P, CPT, 2, F] layout so
each partition's DMA read per tile is one contiguous 16.4 KB burst.

Segment counts are host metadata (np.diff of searchsorted boundaries); the
device multiplies the PSUM sums by a per-core reciprocal-count input.

Pad rows (to make every core the same fixed number of chunks) carry
batch_local = -1, which matches no one-hot column and contributes nothing.
"""

import math

import numpy as np

P = 128           # SBUF partitions / rows per chunk
F = 256           # feature dim
G = 1024          # total segments
NCORES = 8
SEG_PER_CORE = G // NCORES   # 128 segments owned by each core
CPT = 8           # chunks coalesced per DMA tile (~2.06 MB per DMA)

_cache: dict[int, object] = {}


def _build(nchunk: int):
    """Build + compile the single-core Bass program (same on all 8 cores)."""
    import concourse.mybir as mybir
    import concourse.tile as tile
    from concourse import bacc

    ntile = nchunk // CPT
    nc = bacc.Bacc("TRN2", target_bir_lowering=False, debug=False)

    bf16 = mybir.dt.bfloat16
    f32 = mybir.dt.float32

    # [ntile*P, CPT, 2, F] bf16: chunk j of tile t at partition p holds the
    # bf16 hi row then the bf16 lo row; 16.4 KB contiguous per partition.
    x = nc.dram_tensor("x", [ntile * P, CPT, 2, F], bf16, kind="ExternalInput").ap()
    b_t = nc.dram_tensor("b_t", [P, nchunk], f32, kind="ExternalInput").ap()
    iota_c = nc.dram_tensor("iota_c", [P, SEG_PER_CORE], f32, kind="ExternalInput").ap()
    recip_c = nc.dram_tensor("recip_c", [SEG_PER_CORE, 1], f32, kind="ExternalInput").ap()
    out = nc.dram_tensor("out", [SEG_PER_CORE, F], f32, kind="ExternalOutput").ap()

    with tile.TileContext(nc) as tc:
        with (
            tc.tile_pool(name="xpool", bufs=4) as xpool,
            tc.tile_pool(name="hotpool", bufs=8) as hotpool,
            tc.tile_pool(name="cpool", bufs=1) as cpool,
            tc.tile_pool(name="opool", bufs=1) as opool,
            tc.tile_pool(name="psum", bufs=1, space="PSUM") as psum_pool,
        ):
            bt_sb = cpool.tile([P, nchunk], f32)
            iota_sb = cpool.tile([P, SEG_PER_CORE], f32)
            recip_sb = cpool.tile([SEG_PER_CORE, 1], f32)

            acc = psum_pool.tile([SEG_PER_CORE, F], f32, space="PSUM")

            for t in range(ntile):
                xt = xpool.tile([P, CPT, 2, F], bf16)
                # alternate the two HWDGE rings (SP and ACT sequencers)
                dma_eng = nc.sync if t % 2 == 0 else nc.scalar
                dma_eng.dma_start(xt[:], x[t * P : (t + 1) * P])
                if t == 0:
                    # constants issue after the first x tile so the big
                    # streaming pipeline starts immediately
                    nc.sync.dma_start(bt_sb[:], b_t[:])
                    nc.sync.dma_start(iota_sb[:], iota_c[:])
                    nc.sync.dma_start(recip_sb[:], recip_c[:])
                for j in range(CPT):
                    c = t * CPT + j
                    hot = hotpool.tile([P, SEG_PER_CORE], bf16)
                    nc.vector.tensor_scalar(
                        out=hot[:],
                        in0=iota_sb[:],
                        scalar1=bt_sb[:, c : c + 1],
                        scalar2=None,
                        op0=mybir.AluOpType.is_equal,
                    )
                    for h in range(2):
                        nc.tensor.matmul(
                            out=acc[:],
                            lhsT=hot[:],
                            rhs=xt[:, j, h, :],
                            start=(c == 0 and h == 0),
                            stop=(c == nchunk - 1 and h == 1),
                        )

            res = opool.tile([SEG_PER_CORE, F], f32)
            nc.vector.tensor_scalar_mul(res[:], acc[:], recip_sb[:])
            nc.sync.dma_start(out[:], res[:])

    nc.compile()
    return nc


def _compiled(nchunk: int):
    if nchunk not in _cache:
        _cache[nchunk] = _build(nchunk)
    return _cache[nchunk]


def make_in_maps(x: np.ndarray, batch: np.ndarray):
    """Host-side shard/pad/layout. Returns (in_maps, nchunk)."""
    import ml_dtypes

    bf16 = ml_dtypes.bfloat16

    x = np.asarray(x, dtype=np.float32)
    batch_i = np.asarray(batch).astype(np.int64, copy=False)
    n = x.shape[0]
    assert x.shape == (n, F) and batch_i.shape == (n,)

    off = np.searchsorted(batch_i, np.arange(G + 1), side="left")
    counts = np.maximum(np.diff(off), 1).astype(np.float32)
    core_off = off[:: SEG_PER_CORE]            # [NCORES + 1] row boundaries
    rows = np.diff(core_off)
    nchunk = math.ceil(rows.max() / P)
    nchunk = ((nchunk + CPT - 1) // CPT) * CPT

    iota_np = np.tile(np.arange(SEG_PER_CORE, dtype=np.float32), (P, 1))

    ntile = nchunk // CPT
    in_maps = []
    for k in range(NCORES):
        s, e = int(core_off[k]), int(core_off[k + 1])
        nreal = e - s
        xs = x[s:e]
        hi = np.zeros((nchunk * P, F), bf16)
        hi[:nreal] = xs.astype(bf16)
        lo = np.zeros((nchunk * P, F), bf16)
        lo[:nreal] = (xs - hi[:nreal].astype(np.float32)).astype(bf16)
        # [nchunk*P, 2, F] -> [ntile, CPT, P, 2, F] -> [ntile, P, CPT, 2, F]
        pair = np.stack([hi, lo], axis=1)
        xarr = np.ascontiguousarray(
            pair.reshape(ntile, CPT, P, 2, F).swapaxes(1, 2)
        ).reshape(ntile * P, CPT, 2, F)
        b = np.full((nchunk * P,), -1.0, np.float32)
        b[:nreal] = (batch_i[s:e] - k * SEG_PER_CORE).astype(np.float32)
        in_maps.append(
            {
                "x": xarr,
                "b_t": np.ascontiguousarray(b.reshape(nchunk, P).T),
                "iota_c": iota_np,
                "recip_c": (1.0 / counts[k * SEG_PER_CORE : (k + 1) * SEG_PER_CORE])
                .astype(np.float32)
                .reshape(-1, 1),
            }
        )
    return in_maps, nchunk


def run_spmd(in_maps, nchunk, **kwargs):
    from concourse.bass_utils import run_bass_kernel_spmd

    nc = _compiled(nchunk)
    return run_bass_kernel_spmd(nc, in_maps, core_ids=list(range(NCORES)), **kwargs)


def kernel(x: np.ndarray, batch: np.ndarray) -> np.ndarray:
    in_maps, nchunk = make_in_maps(x, batch)
    res = run_spmd(in_maps, nchunk)
    return np.concatenate([res.results[k]["out"] for k in range(NCORES)], axis=0)


# revision 9
# speedup vs baseline: 1.3404x; 1.1688x over previous
"""Segment mean-pool (global_mean_pool) kernel for Trainium2, 8 NeuronCores.

Problem: x [1_000_000, 256] f32, batch [1_000_000] sorted int in [0, 1024).
Output [1024, 256]: per-segment mean of rows of x.

Strategy
--------
batch is sorted, so each segment is a contiguous row range. We shard by
*segment boundaries*: core k owns the 128 segments [128k, 128k+128) and the
contiguous rows belonging to them. Each core computes its 128 output rows
fully on-device, so no collective is needed; the host just concatenates the
eight [128, 256] results.

Per core, rows are streamed in 128-row chunks. For each chunk the device
builds a one-hot routing matrix hot[i, s] = (batch_local[row i] == s) with a
single VectorE tensor_scalar(is_equal) against a resident iota constant, and
TensorE accumulates hot.T @ x_chunk into a PSUM tile [128 segs, 256].

Precision/throughput trick: fp32 matmuls cost 4 PE cycles per column. So the
host splits x into a bf16 hi/lo pair (hi = bf16(x), lo = bf16(x - hi); same
4 bytes/element as fp32, ~17 effective mantissa bits), and each chunk does
two bf16 matmuls (1 cycle/column) with the same exact 0/1 one-hot,
accumulating both into the same fp32 PSUM tile. Result matches the fp32
reference to ~1e-6 relative error at 4x the PE throughput of fp32.

The hi/lo pair is interleaved per chunk in a [ntile, P, CPT, 2, F] layout so
each partition's DMA read per tile is one contiguous 16.4 KB burst.

Segment counts are host metadata (np.diff of searchsorted boundaries); the
device multiplies the PSUM sums by a per-core reciprocal-count input.

Pad rows (to make every core the same fixed number of chunks) carry
batch_local = -1, which matches no one-hot column and contributes nothing.
"""

import math

import numpy as np

P = 128           # SBUF partitions / rows per chunk
F = 256           # feature dim
G = 1024          # total segments
NCORES = 8
SEG_PER_CORE = G // NCORES   # 128 segments owned by each core
CPT = 8           # chunks coalesced per DMA tile (~2.06 MB per DMA)

_cache: dict[int, object] = {}


def _build(nchunk: int):
    """Build + compile the single-core Bass program (same on all 8 cores)."""
    import concourse.mybir as mybir
    import concourse.tile as tile
    from concourse import bacc

    ntile = nchunk // CPT
    nc = bacc.Bacc("TRN2", target_bir_lowering=False, debug=False)

    bf16 = mybir.dt.bfloat16
    f32 = mybir.dt.float32

    # [ntile*P, CPT, 2, F] bf16: chunk j of tile t at partition p holds the
    # bf16 hi row then the bf16 lo row; 16.4 KB contiguous per partition.
    x = nc.dram_tensor("x", [ntile * P, CPT, 2, F], bf16, kind="ExternalInput").ap()
    b_t = nc.dram_tensor("b_t", [P, nchunk], f32, kind="ExternalInput").ap()
    iota_c = nc.dram_tensor("iota_c", [P, SEG_PER_CORE], f32, kind="ExternalInput").ap()
    recip_c = nc.dram_tensor("recip_c", [SEG_PER_CORE, 1], f32, kind="ExternalInput").ap()
    out = nc.dram_tensor("out", [SEG_PER_CORE, F], f32, kind="ExternalOutput").ap()

    with tile.TileContext(nc) as tc:
        with (
            tc.tile_pool(name="xpool", bufs=6) as xpool,
            tc.tile_pool(name="hotpool", bufs=8) as hotpool,
            tc.tile_pool(name="cpool", bufs=1) as cpool,
            tc.tile_pool(name="opool", bufs=1) as opool,
            tc.tile_pool(name="psum", bufs=1, space="PSUM") as psum_pool,
        ):
            bt_sb = cpool.tile([P, nchunk], f32)
            iota_sb = cpool.tile([P, SEG_PER_CORE], f32)
            recip_sb = cpool.tile([SEG_PER_CORE, 1], f32)

            acc = psum_pool.tile([SEG_PER_CORE, F], f32, space="PSUM")

            for t in range(ntile):
                xt = xpool.tile([P, CPT, 2, F], bf16)
                nc.sync.dma_start(xt[:], x[t * P : (t + 1) * P])
                if t == 0:
                    # constants issue after the first x tile so the big
                    # streaming pipeline starts immediately
                    nc.sync.dma_start(bt_sb[:], b_t[:])
                    nc.sync.dma_start(iota_sb[:], iota_c[:])
                    nc.sync.dma_start(recip_sb[:], recip_c[:])
                for j in range(CPT):
                    c = t * CPT + j
                    hot = hotpool.tile([P, SEG_PER_CORE], bf16)
                    nc.vector.tensor_scalar(
                        out=hot[:],
                        in0=iota_sb[:],
                        scalar1=bt_sb[:, c : c + 1],
                        scalar2=None,
                        op0=mybir.AluOpType.is_equal,
                    )
                    for h in range(2):
                        nc.tensor.matmul(
                            out=acc[:],
                            lhsT=hot[:],
                            rhs=xt[:, j, h, :],
                            start=(c == 0 and h == 0),
                            stop=(c == nchunk - 1 and h == 1),
                        )

            res = opool.tile([SEG_PER_CORE, F], f32)
            nc.vector.tensor_scalar_mul(res[:], acc[:], recip_sb[:])
            nc.sync.dma_start(out[:], res[:])

    nc.compile()
    return nc


def _compiled(nchunk: int):
    if nchunk not in _cache:
        _cache[nchunk] = _build(nchunk)
    return _cache[nchunk]


def make_in_maps(x: np.ndarray, batch: np.ndarray):
    """Host-side shard/pad/layout. Returns (in_maps, nchunk)."""
    import ml_dtypes

    bf16 = ml_dtypes.bfloat16

    x = np.asarray(x, dtype=np.float32)
    batch_i = np.asarray(batch).astype(np.int64, copy=False)
    n = x.shape[0]
    assert x.shape == (n, F) and batch_i.shape == (n,)

    off = np.searchsorted(batch_i, np.arange(G + 1), side="left")
    counts = np.maximum(np.diff(off), 1).astype(np.float32)
    core_off = off[:: SEG_PER_CORE]            # [NCORES + 1] row boundaries
    rows = np.diff(core_off)
    nchunk = math.ceil(rows.max() / P)
    nchunk = ((nchunk + CPT - 1) // CPT) * CPT

    iota_np = np.tile(np.arange(SEG_PER_CORE, dtype=np.float32), (P, 1))

    ntile = nchunk // CPT
    in_maps = []
    for k in range(NCORES):
        s, e = int(core_off[k]), int(core_off[k + 1])
        nreal = e - s
        xs = x[s:e]
        hi = np.zeros((nchunk * P, F), bf16)
        hi[:nreal] = xs.astype(bf16)
        lo = np.zeros((nchunk * P, F), bf16)
        lo[:nreal] = (xs - hi[:nreal].astype(np.float32)).astype(bf16)
        # [nchunk*P, 2, F] -> [ntile, CPT, P, 2, F] -> [ntile, P, CPT, 2, F]
        pair = np.stack([hi, lo], axis=1)
        xarr = np.ascontiguousarray(
            pair.reshape(ntile, CPT, P, 2, F).swapaxes(1, 2)
        ).reshape(ntile * P, CPT, 2, F)
        b = np.full((nchunk * P,), -1.0, np.float32)
        b[:nreal] = (batch_i[s:e] - k * SEG_PER_CORE).astype(np.float32)
        in_maps.append(
            {
                "x": xarr,
                "b_t": np.ascontiguousarray(b.reshape(nchunk, P).T),
                "iota_c": iota_np,
                "recip_c": (1.0 / counts[k * SEG_PER_CORE : (k + 1) * SEG_PER_CORE])
                .astype(np.float32)
                .reshape(-1, 1),
            }
        )
    return in_maps, nchunk


def run_spmd(in_maps, nchunk, **kwargs):
    from concourse.bass_utils import run_bass_kernel_spmd

    nc = _compiled(nchunk)
    return run_bass_kernel_spmd(nc, in_maps, core_ids=list(range(NCORES)), **kwargs)


def kernel(x: np.ndarray, batch: np.ndarray) -> np.ndarray:
    in_maps, nchunk = make_in_maps(x, batch)
    res = run_spmd(in_maps, nchunk)
    return np.concatenate([res.results[k]["out"] for k in range(NCORES)], axis=0)
